# revision 1
# baseline (speedup 1.0000x reference)
"""BitLinear (ternary weight + per-token int8 absmax activation) on 8 trn2 cores.

y = (round(x/s) clipped) * s  @  (alpha * clip(round(W/alpha),-1,1)).T  + bias
  with s = max(absmax(x, -1), eps)/127 per token, alpha = max(mean|W|, eps).

Strategy: data-parallel over tokens (4096 tokens/core).  Weight prep is
sharded: each core ternarizes its 256-row slice of W (host-sliced input),
alpha partials are AllReduced, and the transposed ternary weight (bf16) is
AllGathered so every core holds the full W^T for its token matmuls.

Quantized activations are integers in [-127,127] and ternary weights are
{-1,0,1} -- both exact in bf16, and PE accumulates in fp32, so the matmul
integer part is EXACT.  Per-token scale (s*alpha) is applied to the PSUM
result, bias added, all in fp32.  Rounding uses the magic-number trick
(v + 1.5*2^23 - 1.5*2^23) = IEEE round-to-nearest-even == jnp.round.

Tokens are processed in supertiles of ST*128 so DMA transfers are >=2 MiB
and instruction counts stay low; activation transposes ride the ACT HWDGE
ring, separate from the copy ring.
"""

import numpy as np
from contextlib import ExitStack

import concourse.bass as bass
from concourse import bacc
import concourse.mybir as mybir
import concourse.tile as tile
from concourse.bass import ts
from concourse.bass_utils import run_bass_kernel_spmd
from concourse.masks import make_identity

P = 128
D_IN = 2048
D_OUT = 2048
KC = D_IN // P          # 16 contraction chunks
NFREE = 512             # matmul free dim (one PSUM bank of f32)
NT = D_OUT // NFREE     # 4 n-chunks
MAGIC = 12582912.0      # 1.5 * 2**23 : fp32 RNE rounding offset
EPS = 1e-5
CLAMP = float(np.nextafter(np.float32(1.5), np.float32(0.0)))  # largest f32 < 1.5
N_CORES = 8
WS_ROWS = D_OUT // N_CORES          # 256 weight rows per core
WS_CH = WS_ROWS // P                # 2 chunks of 128 rows per core
ST = 2                              # token tiles per supertile
Y_VIA_SWDGE = False                 # y stores on gpsimd (SWDGE) ring
BIAS_ON_POOL = False                # bias add on GpSimd instead of DVE
FINE_GRAIN = False                  # per-subtile transposes + split psum tags
BIG_N = False                       # N=1024 matmuls (2 PSUM banks per MM)

F32 = mybir.dt.float32
BF16 = mybir.dt.bfloat16
Copy = mybir.ActivationFunctionType.Copy
Alu = mybir.AluOpType
AX = mybir.AxisListType
GROUPS = [list(range(N_CORES))]


def _build(T: int, repeat: int = 1) -> bass.Bass:
    """Build the per-core program for T tokens (repeat>1: perf timing only)."""
    st = ST if T % (P * ST) == 0 else 1
    MS = T // (P * st)  # supertiles
    nc = bacc.Bacc(None, target_bir_lowering=False)

    x_d = nc.dram_tensor("x", [T, D_IN], F32, kind="ExternalInput")
    ws_d = nc.dram_tensor("ws", [WS_ROWS, D_IN], F32, kind="ExternalInput")
    b_d = nc.dram_tensor("b", [D_OUT], F32, kind="ExternalInput")
    y_d = nc.dram_tensor("y", [T, D_OUT], F32, kind="ExternalOutput")
    x_v = x_d.rearrange("(s a p) d -> s p a d", p=P, a=st)
    y_v = y_d.rearrange("(s a p) d -> s p a d", p=P, a=st)

    with tile.TileContext(nc) as tc, ExitStack() as ctx:
      const = ctx.enter_context(tc.tile_pool(name="const", bufs=1))
      wload = ctx.enter_context(tc.tile_pool(name="wload", bufs=1))
      wtmp = ctx.enter_context(tc.tile_pool(name="wtmp", bufs=2))
      xin = ctx.enter_context(tc.tile_pool(name="xin", bufs=2))
      xq = ctx.enter_context(tc.tile_pool(name="xq", bufs=2))
      xt = ctx.enter_context(tc.tile_pool(name="xt", bufs=2))
      scl = ctx.enter_context(tc.tile_pool(name="scl", bufs=4))
      yout = ctx.enter_context(tc.tile_pool(name="yout", bufs=2))
      psum = ctx.enter_context(tc.tile_pool(name="psum", bufs=2, space="PSUM"))
      dram = ctx.enter_context(tc.tile_pool(name="dram", bufs=1, space="DRAM"))
      for _rep in range(repeat):
        wT = const.tile([P, KC, D_OUT], BF16)          # full ternary W^T
        bias_bc = const.tile([P, D_OUT], F32)
        ident = const.tile([P, P], F32)
        partial = const.tile([P, WS_CH], F32)
        my_psum = const.tile([P, 1], F32)
        wsum = const.tile([P, 1], F32)
        alpha_sb = const.tile([P, 1], F32)
        inv_alpha = const.tile([P, 1], F32)
        alpha127 = const.tile([P, 1], F32)

        nc.gpsimd.dma_start(out=bias_bc[:], in_=b_d[None, :].to_broadcast((P, D_OUT)))
        make_identity(nc, ident[:])

        # ---- phase W-A: alpha = max(mean|W|, eps), sharded + AllReduce --
        # accuracy matters: the ternary decision boundary sits ~4e-7
        # (relative) from the nearest weight, so sums are grouped small and
        # finished with an explicit pairwise tree (stays ~1 ulp of f64).
        wcs = []
        for c in range(WS_CH):
            wc = wload.tile([P, D_IN], F32, tag=f"wchunk{c}", bufs=1)
            nc.sync.dma_start(out=wc[:], in_=ws_d[ts(c, P), :])
            s1 = scl.tile([P, KC], F32, tag="s1")
            nc.vector.tensor_reduce(
                s1[:], wc.rearrange("p (a b) -> p a b", a=KC), axis=AX.X,
                op=Alu.add, apply_absolute_value=True,
            )
            nc.vector.tensor_reduce(
                partial[:, c : c + 1], s1[:], axis=AX.X, op=Alu.add
            )
            wcs.append(wc)
        nc.vector.tensor_reduce(my_psum[:], partial[:], axis=AX.X, op=Alu.add)
        # AllReduce the per-partition partial sums across cores
        ar_in = dram.tile([P, 1], F32, name="ar_in")
        ar_out = dram.tile([P, 1], F32, name="ar_out", addr_space="Shared")
        nc.sync.dma_start(out=ar_in[:], in_=my_psum[:])
        nc.gpsimd.collective_compute(
            "AllReduce", Alu.add, replica_groups=GROUPS,
            ins=[ar_in[:]], outs=[ar_out[:]],
        )
        nc.sync.dma_start(out=wsum[:], in_=ar_out[:])
        # 128 per-partition totals -> one row (exact PE transpose), then a
        # pairwise tree of 7 adds.
        ps_t = psum.tile([1, P], F32, tag="psf0" if FINE_GRAIN else "ps", bufs=1 if FINE_GRAIN else 2)
        nc.tensor.transpose(ps_t[:], wsum[:], ident[:])
        row = const.tile([1, P], F32)
        nc.scalar.copy(row[:], ps_t[:])
        width = P // 2
        while width >= 1:
            nc.vector.tensor_tensor(
                row[0:1, 0:width], row[0:1, 0:width],
                row[0:1, width : 2 * width], op=Alu.add,
            )
            width //= 2
        al_sc = const.tile([1, 1], F32)
        nc.vector.tensor_scalar(
            al_sc[:], row[0:1, 0:1], 1.0 / (D_IN * D_OUT), EPS,
            op0=Alu.mult, op1=Alu.max,
        )
        # broadcast alpha to all partitions through a DRAM bounce
        al_d = dram.tile([1, 1], F32, name="al_d")
        nc.sync.dma_start(out=al_d[:], in_=al_sc[:])
        nc.gpsimd.dma_start(out=alpha_sb[:], in_=al_d[:].to_broadcast((P, 1)))
        nc.vector.reciprocal(inv_alpha[:], alpha_sb[:])
        nc.scalar.mul(alpha127[:], alpha_sb[:], 1.0 / 127.0)

        # ---- phase W-B: ternarize own shard + transpose + AllGather -----
        contrib = dram.tile([P, KC, WS_ROWS], BF16, name="contrib")
        gathered = dram.tile([N_CORES, P, KC, WS_ROWS], BF16, name="gathered",
                             addr_space="Shared")
        for c in range(WS_CH):
            nc.scalar.activation(wcs[c][:], wcs[c][:], Copy, scale=inv_alpha[:])
            # clamp to (-1.5, 1.5) so round gives {-1,0,1} (== clip(round,-1,1))
            nc.gpsimd.tensor_scalar(
                wcs[c][:], wcs[c][:], CLAMP, -CLAMP, op0=Alu.min, op1=Alu.max
            )
            wt = wtmp.tile([P, D_IN], BF16, tag="wtern")
            nc.gpsimd.tensor_scalar(
                wt[:], wcs[c][:], MAGIC, MAGIC, op0=Alu.add, op1=Alu.subtract
            )
            wtl = wtmp.tile([P, KC, P], BF16, tag="wtl", bufs=2)
            nc.scalar.dma_start_transpose(wtl[:], wt[:])
            nc.sync.dma_start(out=contrib[:, :, ts(c, P)], in_=wtl[:])
        nc.gpsimd.collective_compute(
            "AllGather", Alu.bypass, replica_groups=GROUPS,
            ins=[contrib[:]], outs=[gathered[:]],
        )
        for c in range(N_CORES):
            nc.sync.dma_start(out=wT[:, :, ts(c, WS_ROWS)], in_=gathered[c])

        # ---- main token loop: supertiles of st*128 tokens ---------------
        for m in range(MS):
            x_t = xin.tile([P, st, D_IN], F32, tag="x")
            nc.sync.dma_start(out=x_t[:], in_=x_v[m])

            absmax = scl.tile([P, st], F32, tag="absmax")
            m1 = scl.tile([P, st], F32, tag="m1")
            r = scl.tile([P, st], F32, tag="r")
            inv127 = scl.tile([P, st], F32, tag="inv127")
            c_vec = scl.tile([P, st], F32, tag="c_vec")

            nc.vector.tensor_reduce(
                absmax[:], x_t[:], axis=AX.X, op=Alu.max, apply_absolute_value=True
            )
            nc.vector.tensor_scalar(m1[:], absmax[:], EPS, None, op0=Alu.max)
            nc.vector.reciprocal(r[:], m1[:])
            nc.scalar.mul(inv127[:], r[:], 127.0)
            nc.scalar.mul(c_vec[:], m1[:], alpha127[:])

            # q = round(x * 127/m1)  as bf16 integers
            for a in range(st):
                nc.scalar.activation(
                    x_t[:, a, :], x_t[:, a, :], Copy, bias=MAGIC,
                    scale=inv127[:, a : a + 1],
                )
            q_t = xq.tile([P, st, D_IN], BF16, tag="q")
            nc.vector.tensor_scalar(q_t[:], x_t[:], MAGIC, None, op0=Alu.subtract)

            # transpose to [i, t] layout for the matmul (ACT HWDGE ring)
            if FINE_GRAIN:
                xT_subs = []
                for a in range(st):
                    xT_a = xt.tile([P, KC, P], BF16, tag=f"xTf{a}", name=f"xT{a}")
                    nc.scalar.dma_start_transpose(xT_a[:], q_t[:, a, :])
                    xT_subs.append(xT_a)
            else:
                xT_t = xt.tile([P, st * KC, P], BF16, tag="xT")
                nc.scalar.dma_start_transpose(
                    xT_t[:], q_t.rearrange("p a d -> p (a d)"))

            y_t = yout.tile([P, st, D_OUT], F32, tag="y")
            for a in range(st):
                if FINE_GRAIN:
                    ps = psum.tile([P, NT, NFREE], F32, tag=f"psf{a}", name="ps",
                                   bufs=1)
                    lhs = xT_subs[a]
                    lhs_base = 0
                else:
                    ps = psum.tile([P, NT, NFREE], F32, tag="ps", name="ps")
                    lhs = xT_t
                    lhs_base = a * KC
                if BIG_N:
                    ps2 = ps.rearrange("p a b -> p (a b)")
                    for k in range(KC):
                        for n in range(2):
                            nc.tensor.matmul(
                                ps2[:, ts(n, 1024)],
                                lhs[:, lhs_base + k, :],
                                wT[:, k, ts(n, 1024)],
                                start=(k == 0),
                                stop=(k == KC - 1),
                            )
                elif m == 0:
                    # first supertile: n-outer so the n=0 group only needs
                    # the first gather slices -- PE starts before the full
                    # W^T staging lands
                    for n in range(NT):
                        for k in range(KC):
                            nc.tensor.matmul(
                                ps[:, n, :],
                                lhs[:, lhs_base + k, :],
                                wT[:, k, ts(n, NFREE)],
                                start=(k == 0),
                                stop=(k == KC - 1),
                            )
                else:
                    for k in range(KC):
                        for n in range(NT):
                            nc.tensor.matmul(
                                ps[:, n, :],
                                lhs[:, lhs_base + k, :],
                                wT[:, k, ts(n, NFREE)],
                                start=(k == 0),
                                stop=(k == KC - 1),
                            )
                ps_flat = ps.rearrange("p a b -> p (a b)")
                nc.scalar.activation(
                    y_t[:, a, :], ps_flat, Copy, scale=c_vec[:, a : a + 1]
                )
            bias_eng = nc.gpsimd if BIAS_ON_POOL else nc.vector
            bias_eng.tensor_tensor(
                y_t[:], y_t[:],
                bias_bc[:, None, :].to_broadcast((P, st, D_OUT)), op=Alu.add,
            )
            y_eng = nc.gpsimd if Y_VIA_SWDGE else nc.sync
            y_eng.dma_start(out=y_v[m], in_=y_t[:])

    nc.compile()
    return nc


_PROG_CACHE: dict[tuple, bass.Bass] = {}


def _get_prog(T: int, repeat: int = 1) -> bass.Bass:
    key = (T, repeat)
    if key not in _PROG_CACHE:
        _PROG_CACHE[key] = _build(T, repeat)
    return _PROG_CACHE[key]


def _make_in_maps(xf: np.ndarray, w: np.ndarray, b: np.ndarray, T: int):
    st = ST if T % (P * ST) == 0 else 1
    return [
        {
            "x": np.ascontiguousarray(xf[c * T : (c + 1) * T]),
            "ws": np.ascontiguousarray(w[c * WS_ROWS : (c + 1) * WS_ROWS]),
            "b": b,
        }
        for c in range(N_CORES)
    ]


def kernel(x: np.ndarray, weight: np.ndarray, bias: np.ndarray) -> np.ndarray:
    orig_shape = x.shape
    xf = np.ascontiguousarray(x.reshape(-1, D_IN).astype(np.float32, copy=False))
    n_tok = xf.shape[0]
    assert n_tok % N_CORES == 0
    T = n_tok // N_CORES
    w = np.ascontiguousarray(weight.astype(np.float32, copy=False))
    b = np.ascontiguousarray(bias.astype(np.float32, copy=False))

    nc = _get_prog(T)
    in_maps = _make_in_maps(xf, w, b, T)
    res = run_bass_kernel_spmd(nc, in_maps, core_ids=list(range(N_CORES)))
    y = np.concatenate([r["y"] for r in res.results], axis=0)
    return y.reshape(orig_shape[:-1] + (D_OUT,)).astype(np.float32)

